# revision 1
# baseline (speedup 1.0000x reference)
"""Trainium2 Bass kernel for the DifferentiableLayer (moe_routing) problem.

Computes out[b, o] = sum_{i,k} onehot(argmax_k(weights+gumbel))[o,i,k] * ops(x)[b,i,k]
where ops(x) = [x, sin x, cos x, tanh x, x^2, relu x] along k.

Forward value of the straight-through hard gumbel-softmax is exactly the
one-hot of argmax_k(weights + gumbel) (softmax is monotonic), so per core:
  1. s = w + g   (w DMA'd, g DMA'd with accum_op=add -> fp32 exact)
  2. m = max_k s, one-hot P_k = (s_k == m) via a zero-stride broadcast
     compare, pipelined in (t, h) chunks as the w/g DMAs land  [VectorE]
  3. x -> bf16; x^T and P^T via identity-matmuls on the tensor engine
     (DMA-xbar transpose stalls behind bulk copies, so PE does it)
  4. ops(x^T): ScalarE sin/cos (range-folded), tanh, |x|; VectorE square+relu
       sin(x) = Sin(2pi*[x>=0] - x - pi)   (arg in [-pi,pi] for |x| <= 2pi)
       cos(x) = Sin(2pi*[|x|<=pi/2] + |x| - 3pi/2)
  5. out^T[o,b] = sum_{k,i} P^T . ops^T -- 96 accumulating bf16 matmuls
Sharding: 4 batch shards x 2 out-feature shards over 8 cores.

The 64-byte engine instruction structs have a single sync-wait slot, so
cross-engine waits that would stack on one instruction are absorbed by
dependency-carrying nops, and a post-pass strips waits that are provably
dominated by an earlier wait on the same in-order queue.
"""

import numpy as np

from concourse import bass, mybir, tile
from concourse.bass import _add_dep_helper
from concourse.bass_utils import run_bass_kernel_spmd

F32 = mybir.dt.float32
BF16 = mybir.dt.bfloat16
AF = mybir.ActivationFunctionType
ALU = mybir.AluOpType

B, I, O, K = 4096, 512, 512, 6
NB, NO = 4, 2                # batch shards x out-feature shards
BL, OL = B // NB, O // NO    # 1024, 256 per core
NCORES = NB * NO

NJ = BL // 128               # 8 b-tiles
NT = OL // 128               # 2 o-tiles
NIT = I // 128               # 4 i-tiles
IK = I * K                   # 3072 contraction size
IH = I // 2                  # 256: i-half for (t,h) pipeline chunks
PTR_BUFS = 4

_ENGINE_SEM = {
    "EngineType.PE": "PE",
    "EngineType.Activation": "Activation",
    "EngineType.DVE": "DVE",
}


def _strip_redundant_waits(nc: bass.Bass) -> None:
    """Drop sync waits that are dominated by an earlier wait on the same
    in-order engine queue, or (for PE/ACT/DVE) implied by the engine's own
    completion-semaphore order.  Needed because the HW instruction structs
    hold a single wait command."""
    import re

    seen = {}      # sem name -> cumulative update count
    observed = {}  # (engine, sem name) -> max wait value already waited for
    for bb in nc.main_func.blocks:
        for ins in bb.instructions:
            si = ins.sync_info
            if si is None:
                continue
            eng = str(ins.engine)
            if len(si.on_wait) >= 2:
                own = _ENGINE_SEM.get(eng)
                keep = []
                for w in si.on_wait:
                    if observed.get((eng, w.ant_name), -1) >= w.wait_value:
                        continue
                    if (
                        own is not None
                        and re.fullmatch(rf"{own}_\d+", w.ant_name)
                        and seen.get(w.ant_name, 0) >= w.wait_value
                    ):
                        continue
                    keep.append(w)
                if len(keep) != len(si.on_wait):
                    si.on_wait = keep
            for w in si.on_wait:
                key = (eng, w.ant_name)
                if observed.get(key, -1) < w.wait_value:
                    observed[key] = w.wait_value
            for u in si.on_update:
                if u.update_value is not None:
                    seen[u.ant_name] = seen.get(u.ant_name, 0) + u.update_value


def _build_program() -> bass.Bass:
    nc = bass.Bass()

    x_in = nc.declare_dram_parameter("x", [BL, I], F32, isOutput=False)
    w_in = nc.declare_dram_parameter("w", [OL, I, K], F32, isOutput=False)
    g_in = nc.declare_dram_parameter("g", [OL, I, K], F32, isOutput=False)
    out_ext = nc.declare_dram_parameter("out", [OL, BL], F32, isOutput=True)

    def dep(a, b, why):
        _add_dep_helper(a.ins, b.ins, sync=True, reason=why)

    with tile.TileContext(nc) as tc:
        with (
            tc.tile_pool(name="const", bufs=1) as constp,
            tc.tile_pool(name="big", bufs=1) as big,
            tc.tile_pool(name="psum_tr", bufs=1, space="PSUM") as ptr,
            tc.tile_pool(name="psum_out", bufs=1, space="PSUM") as pout,
        ):
            ident = constp.tile([128, 128], BF16)
            id_ms = nc.gpsimd.memset(ident[:], 0.0)
            id_aff = nc.gpsimd.affine_select(
                out=ident[:], in_=ident[:], compare_op=ALU.not_equal,
                fill=1.0, base=0, pattern=[[-1, 128]], channel_multiplier=1,
            )
            ident_ready = nc.gpsimd.nop()
            dep(ident_ready, id_aff, "identity ready marker")
            neg_pi_sb = constp.tile([128, 1], F32)
            negpi_ins = nc.gpsimd.memset(neg_pi_sb[:], -float(np.pi))
            neg_3pi2_sb = constp.tile([128, 1], F32)
            negpi32_ins = nc.gpsimd.memset(neg_3pi2_sb[:], -float(1.5 * np.pi))
            act_r1 = nc.scalar.nop()
            dep(act_r1, negpi_ins, "absorb const memset wait on ACT")
            act_r2 = nc.scalar.nop()
            dep(act_r2, negpi32_ins, "absorb const memset wait on ACT")

            # ---- SBUF tiles ----
            x_sb = big.tile([128, NJ * I], F32)        # [p=b%128, (j, i)]
            xb_sb = big.tile([128, NJ * I], BF16)
            xT_sb = big.tile([128, NJ * I], BF16)      # [p=i%128, (it, b)]
            w_sb = big.tile([128, NT * IK], F32)       # becomes s = w + g
            m_sb = big.tile([128, NT * I], F32)        # [p=o%128, (t, h, i')]
            p_sb = big.tile([128, NT * K * I], BF16)   # [p=o%128, (t, h, k, i')]
            pT_sb = big.tile([128, NT * K * I], BF16)  # [p=i%128, (t,h,k,it2,o)]
            tr1_sb = big.tile([128, NJ * I], BF16)
            tr2_sb = big.tile([128, NJ * I], BF16)
            tr3_sb = big.tile([128, NJ * I], BF16)
            ops_sb = big.tile([128, 5 * NJ * I], BF16)  # [p, (q, it, b)]
            out_sb = big.tile([128, NT * BL], F32)     # [p=o%128, (t, b)]

            x_v = x_sb[:].rearrange("p (j i) -> p j i", j=NJ)
            xb_v = xb_sb[:].rearrange("p (j i) -> p j i", j=NJ)
            xT_v = xT_sb[:].rearrange("p (it b) -> p it b", it=NIT)
            w_v = w_sb[:].rearrange("p (t h ik) -> p t h ik", t=NT, h=2)
            s_hik = w_sb[:].rearrange(
                "p (t h i k) -> p t h i k", t=NT, h=2, k=K
            )
            m_v = m_sb[:].rearrange("p (t h i) -> p t h i", t=NT, h=2)
            m_t = m_sb[:].rearrange("p (t i) -> p t i", t=NT)
            s_ik = w_sb[:].rearrange("p (t i k) -> p t i k", t=NT, k=K)
            p_tk = p_sb[:].rearrange("p (t k i) -> p t k i", t=NT, k=K)
            pT_tk = pT_sb[:].rearrange("p (t k ito) -> p t k ito", t=NT, k=K)
            ops_f = ops_sb[:].rearrange("p (q itb) -> p q itb", q=5)
            ops_v = ops_sb[:].rearrange("p (q it b) -> p q it b", q=5, it=NIT)
            out_v = out_sb[:].rearrange("p (t b) -> p t b", t=NT)

            # ---- loads ----
            tail_deps = [id_ms, id_aff, negpi_ins, negpi32_ins]
            x_dram = x_in[:].rearrange("(j p) i -> p j i", p=128)
            for jh in range(2):
                xd = nc.sync.dma_start(
                    out=x_v[:, jh * 4 : (jh + 1) * 4],
                    in_=x_dram[:, jh * 4 : (jh + 1) * 4],
                )
                tail_deps.append(xd)
                nc.vector.tensor_copy(
                    xb_v[:, jh * 4 : (jh + 1) * 4], x_v[:, jh * 4 : (jh + 1) * 4]
                )

            w_dram = w_in[:].rearrange("(t p) i k -> p t (i k)", p=128).rearrange(
                "p t (h ik) -> p t h ik", h=2
            )
            g_dram = g_in[:].rearrange("(t p) i k -> p t (i k)", p=128).rearrange(
                "p t (h ik) -> p t h ik", h=2
            )
            g_accums = {}
            for t in range(NT):
                for h in range(2):
                    wd = nc.sync.dma_start(out=w_v[:, t, h], in_=w_dram[:, t, h])
                    tail_deps.append(wd)
                    ng = nc.gpsimd.nop()
                    dep(ng, wd, "absorb w dma wait before g accum")
                    ga = nc.gpsimd.dma_start(
                        out=w_v[:, t, h], in_=g_dram[:, t, h],
                        accum_op=ALU.add,
                    )
                    g_accums[(t, h)] = ga
                    tail_deps.append(ga)
                    tail_deps.append(ng)

            # ---- transpose machinery: psum staging banks on PE ----
            banks = []
            for i in range(PTR_BUFS):
                tr_bank = ptr.tile([128, 512], F32, tag=f"trb{i}")
                banks.append(tr_bank)
            drains = []
            state = {"tenancy": 0, "first_pe": True}

            def start_tenancy():
                if state["first_pe"]:
                    n0 = nc.tensor.nop()
                    dep(n0, ident_ready, "absorb ident wait on PE")
                    state["first_pe"] = False
                i = state["tenancy"]
                if i >= PTR_BUFS:
                    n = nc.tensor.nop()
                    dep(n, drains[i - PTR_BUFS], "absorb psum WAR wait")
                state["tenancy"] += 1
                return banks[i % PTR_BUFS]

            # ---- x^T via identity matmuls ----
            for it in range(NIT):
                for jh in range(2):
                    bank = start_tenancy()
                    for jj in range(4):
                        j = jh * 4 + jj
                        nc.tensor.matmul(
                            bank[:, jj * 128 : (jj + 1) * 128],
                            xb_v[:, j, it * 128 : (it + 1) * 128],
                            ident[:],
                            start=True,
                            stop=True,
                        )
                    d = nc.scalar.copy(
                        xT_v[:, it, jh * 512 : (jh + 1) * 512], bank[:]
                    )
                    drains.append(d)

            # ---- ops on x^T ----
            two_pi = float(2 * np.pi)
            half_pi = float(np.pi / 2)
            xT_all = xT_sb[:]
            nc.vector.tensor_scalar(
                tr1_sb[:], xT_all, 0.0, two_pi, op0=ALU.is_ge, op1=ALU.mult
            )
            relu_ins = nc.vector.tensor_scalar_max(ops_f[:, 4], xT_all, 0.0)
            nc.vector.tensor_sub(tr1_sb[:], tr1_sb[:], xT_all)
            sin_ins = nc.scalar.activation(
                ops_f[:, 0], tr1_sb[:], AF.Sin, bias=neg_pi_sb[:]
            )
            # |x| = 2*relu(x) - x, reusing the already-computed relu
            nc.vector.scalar_tensor_tensor(
                tr2_sb[:], ops_f[:, 4], 2.0, xT_all,
                op0=ALU.mult, op1=ALU.subtract,
            )
            nc.vector.tensor_scalar(
                tr3_sb[:], tr2_sb[:], half_pi, two_pi, op0=ALU.is_le, op1=ALU.mult
            )
            nc.vector.tensor_add(tr3_sb[:], tr3_sb[:], tr2_sb[:])
            cos_ins = nc.scalar.activation(
                ops_f[:, 1], tr3_sb[:], AF.Sin, bias=neg_3pi2_sb[:]
            )
            tanh_ins = nc.scalar.activation(ops_f[:, 2], xT_all, AF.Tanh)
            sq_ins = nc.vector.tensor_mul(ops_f[:, 3], xT_all, xT_all)
            op_dve = {4: sq_ins, 5: relu_ins}

            # ---- selection one-hot + P^T + main matmuls per o-tile t ----
            po = []
            for i in range(4):
                po_tile = pout.tile([128, 512], F32, tag=f"po{i}")
                po.append(po_tile)

            out_dram = out_ext[:].rearrange("(t p) b -> p t b", p=128)
            for t in range(NT):
                for h in range(2):
                    nv = nc.vector.nop()
                    dep(nv, g_accums[(t, h)], "absorb g accum wait on DVE")
                    tail_deps.append(nv)
                    nc.vector.tensor_reduce(
                        m_v[:, t, h], s_hik[:, t, h],
                        axis=mybir.AxisListType.X, op=ALU.max,
                    )
                # one-hot per (t, k): strided s reads, contiguous P writes
                for k in range(K):
                    nc.vector.tensor_tensor(
                        p_tk[:, t, k], s_ik[:, t, :, k], m_t[:, t],
                        op=ALU.is_equal,
                    )
                # P^T: 6 banks of 4 identity-matmul transposes
                for k in range(K):
                    bank = start_tenancy()
                    for it in range(NIT):
                        nc.tensor.matmul(
                            bank[:, it * 128 : (it + 1) * 128],
                            p_tk[:, t, k, it * 128 : (it + 1) * 128],
                            ident[:],
                            start=True,
                            stop=True,
                        )
                    d = nc.scalar.copy(pT_tk[:, t, k], bank[:])
                    drains.append(d)

                absorbed = set()
                for k in range(K):
                    if k in op_dve and id(op_dve[k]) not in absorbed:
                        n = nc.tensor.nop()
                        dep(n, op_dve[k], "absorb DVE op wait on PE")
                        absorbed.add(id(op_dve[k]))
                    for it in range(NIT):
                        lhsT = pT_tk[:, t, k, it * 128 : (it + 1) * 128]
                        for bc in range(2):
                            if k == 0:
                                rhs = xT_v[:, it, bc * 512 : (bc + 1) * 512]
                            else:
                                rhs = ops_v[
                                    :, k - 1, it, bc * 512 : (bc + 1) * 512
                                ]
                            last_mm = nc.tensor.matmul(
                                po[t * 2 + bc][:],
                                lhsT,
                                rhs,
                                start=(k == 0 and it == 0),
                                stop=(k == K - 1 and it == NIT - 1),
                            )
                for bc in range(2):
                    nc.scalar.copy(
                        out_v[:, t, bc * 512 : (bc + 1) * 512], po[t * 2 + bc][:]
                    )
                od = nc.sync.dma_start(out=out_dram[:, t], in_=out_v[:, t])
                tail_deps.append(od)

            # absorb all outstanding completions on the SP queue so the
            # framework's tail drain ends up with only dominated waits
            tail_deps.extend(
                [act_r1, act_r2, ident_ready, relu_ins, sq_ins, last_mm,
                 drains[-1]]
            )
            for d in tail_deps:
                n = nc.sync.nop()
                dep(n, d, "tail wait absorb")

    _strip_redundant_waits(nc)
    return nc


_NC_CACHE = None


def _get_program():
    global _NC_CACHE
    if _NC_CACHE is None:
        _NC_CACHE = _build_program()
    return _NC_CACHE


def _shard_inputs(x, weights, gumbel):
    x = np.ascontiguousarray(np.asarray(x, dtype=np.float32))
    w = np.ascontiguousarray(np.asarray(weights, dtype=np.float32))
    g = np.ascontiguousarray(np.asarray(gumbel, dtype=np.float32))
    in_maps = []
    for c in range(NCORES):
        t, bs = divmod(c, NB)
        in_maps.append(
            {
                "x": x[bs * BL : (bs + 1) * BL],
                "w": w[t * OL : (t + 1) * OL],
                "g": g[t * OL : (t + 1) * OL],
            }
        )
    return in_maps


def _unshard(results):
    out = np.empty((B, O), dtype=np.float32)
    for c in range(NCORES):
        t, bs = divmod(c, NB)
        out[bs * BL : (bs + 1) * BL, t * OL : (t + 1) * OL] = results[c]["out"].T
    return out


def kernel(x, weights, gumbel):
    nc = _get_program()
    in_maps = _shard_inputs(x, weights, gumbel)
    res = run_bass_kernel_spmd(nc, in_maps, list(range(NCORES)))
    return _unshard(res.results)


def kernel_traced(x, weights, gumbel, **trace_kwargs):
    """Like kernel() but with profiling; returns (out, BassKernelResults)."""
    nc = _get_program()
    in_maps = _shard_inputs(x, weights, gumbel)
    res = run_bass_kernel_spmd(
        nc, in_maps, list(range(NCORES)), trace=True, **trace_kwargs
    )
    return _unshard(res.results), res

